# revision 35
# baseline (speedup 1.0000x reference)
"""MiniMaxText01 linear attention layer on 8 trn2 NeuronCores.

Strategy: tensor-parallel over heads (4 heads/core). Per core, two phases
through internal DRAM (SBUF can't hold all weights + full-seq activations):

  phase A: qT/kT/v/gT projections as bf16 matmuls (full PE rate), silu on
           ACT for q/k/v, gate stored as PRE-activation (Copy) so phase A
           only ever uses the silu act-table and phase B only the sigmoid
           table (one table switch per kernel, not per iteration).
           hc-outer accumulation order with 4 PSUM groups in flight so the
           first matmul starts ~3us in, paced by the streaming weight DMAs.
  phase B: blocked lightning attention (BLOCK=256) + sigmoid gating + out
           projection. kv state [d,128] per head stays fp32r in SBUF; all
           big matmul operands are bf16 (rel err ~3e-3 vs 2e-2 budget).
           The RMSNorm rsqrt(var) is a per-token scalar that commutes with
           the out projection, so each core emits
             pout = (gate * attn * norm_w) @ w_out        [4096, 2048]
             ssq  = sum over this core's channels of attn^2   [1, 4096]
           and the host applies out = sum_c(pout) * rsqrt(sum_c(ssq)/4096+eps).
"""
import math
import numpy as np
import ml_dtypes
from contextlib import ExitStack

import concourse.bass as bass
import concourse.tile as tile
import concourse.mybir as mybir
from concourse import bacc
from concourse.bass_utils import run_bass_kernel_spmd

FP32 = mybir.dt.float32
FP32R = mybir.dt.float32r
BF16 = mybir.dt.bfloat16
AF = mybir.ActivationFunctionType

SEQ = 4096
HIDDEN = 2048
NUM_HEADS = 32
HEAD_DIM = 128
INNER = NUM_HEADS * HEAD_DIM
BLOCK = 256
EPS = 1e-5
N_CORES = 8
HPC = NUM_HEADS // N_CORES          # 4 heads per core
IN_PC = HPC * HEAD_DIM              # 512 inner channels per core
P = 128

CH_A = 512
NT_A = SEQ // CH_A                  # 8
CH_B = 512
NT_B = SEQ // CH_B                  # 8
HC = HIDDEN // P                    # 16 hidden chunks
QW = 4                              # xt/weight quarters per chunk


def build_nc(repeat: int = 1, phases: str = "AB", nA: int = 1, nB: int = 1,
             timing: bool = False):
    nc = bacc.Bacc("TRN2", target_bir_lowering=False)

    xt_d = nc.dram_tensor("xt", [HIDDEN, SEQ], BF16, kind="ExternalInput")
    wq_d = nc.dram_tensor("wq", [HIDDEN, IN_PC], BF16, kind="ExternalInput")
    wk_d = nc.dram_tensor("wk", [HIDDEN, IN_PC], BF16, kind="ExternalInput")
    wv_d = nc.dram_tensor("wv", [HIDDEN, IN_PC], BF16, kind="ExternalInput")
    wg_d = nc.dram_tensor("wg", [HIDDEN, IN_PC], BF16, kind="ExternalInput")
    wo_d = nc.dram_tensor("wo", [IN_PC, HIDDEN], BF16, kind="ExternalInput")
    qdec_d = nc.dram_tensor("qdec", [HPC, P, CH_B], BF16, kind="ExternalInput")
    dmask_d = nc.dram_tensor("dmask", [HPC, P, 384], BF16, kind="ExternalInput")
    kdv_d = nc.dram_tensor("kdv", [P, 4, HPC, P], BF16, kind="ExternalInput")
    bdI_d = nc.dram_tensor("bdI", [P, HPC, P], FP32, kind="ExternalInput")
    ones_d = nc.dram_tensor("ones", [P, 1], FP32, kind="ExternalInput")
    kv0_d = nc.dram_tensor("kv0", [HPC, P, P], FP32, kind="ExternalInput")

    # timing builds keep the full compute + DMA but only ship a tiny output
    okind = "Internal" if timing else "ExternalOutput"
    pout_d = nc.dram_tensor("pout", [SEQ, HIDDEN], BF16, kind=okind)
    ssq_d = nc.dram_tensor("ssq", [1, SEQ], FP32, kind=okind)
    if timing:
        tout_d = nc.dram_tensor("tout", [1, 4], FP32, kind="ExternalOutput")

    # internal DRAM intermediates (bf16 round trip)
    qT_d = nc.dram_tensor("qT_i", [IN_PC, SEQ], BF16)
    kT_d = nc.dram_tensor("kT_i", [IN_PC, SEQ], BF16)
    v_d = nc.dram_tensor("v_i", [SEQ, IN_PC], BF16)
    gT_d = nc.dram_tensor("gT_i", [IN_PC, SEQ], BF16)   # gate PRE-activation

    with tile.TileContext(nc) as tc, ExitStack() as ctx:
        const = ctx.enter_context(tc.tile_pool(name="const", bufs=1))
        sB = ctx.enter_context(tc.tile_pool(name="sB", bufs=1))
        gx = ctx.enter_context(tc.tile_pool(name="gx", bufs=1))

        qT_r = qT_d[:].rearrange("(h p) n -> p h n", p=P)
        kT_r = kT_d[:].rearrange("(h p) n -> p h n", p=P)
        gT_r = gT_d[:].rearrange("(h p) n -> p h n", p=P)

        def load_b_tiles(t, eng=None):
            eng = eng or nc.scalar
            tsl = slice(t * CH_B, (t + 1) * CH_B)
            k_t = sB.tile([P, HPC, CH_B], BF16, name="k_t", tag="k", bufs=2)
            eng.dma_start(k_t[:], kT_r[:, :, tsl])
            q_t = sB.tile([P, HPC, CH_B], BF16, name="q_t", tag="q", bufs=2)
            eng.dma_start(q_t[:], qT_r[:, :, tsl])
            v_t = sB.tile([P, 4, IN_PC], BF16, name="v_t", tag="v", bufs=2)
            eng.dma_start(
                v_t[:], v_d[tsl, :].rearrange("(s p) c -> p s c", p=P))
            # k in natural [token, channel] layout via xbar transpose DMA
            kN_t = sB.tile([P, 4, IN_PC], BF16, name="kN_t", tag="kN", bufs=2)
            for s in range(4):
                eng.dma_start_transpose(
                    kN_t[:, s, :],
                    kT_d[:, t * CH_B + s * P: t * CH_B + (s + 1) * P])
            g_t = sB.tile([P, HPC, CH_B], BF16, name="g_t", tag="g", bufs=2)
            eng.dma_start(g_t[:], gT_r[:, :, tsl])
            gs_t = sB.tile([P, HPC, CH_B], BF16, name="gs_t", tag="gs", bufs=2)
            nc.scalar.activation(gs_t[:], g_t[:], AF.Sigmoid)
            return k_t, q_t, v_t, kN_t, gs_t

        ones_t = const.tile([P, 1], FP32R)
        qdec_t = const.tile([P, HPC, CH_B], BF16)
        dmask_t = const.tile([P, HPC, 384], BF16)
        kdv_t = const.tile([P, 4, HPC, P], BF16)
        bdI_t = const.tile([P, HPC, P], FP32R)
        kv_t = const.tile([P, HPC, P], FP32R)
        wo_t = const.tile([P, HPC, HIDDEN], BF16)

        for _rep in range(repeat):
            # ---------------- phase A: projections ----------------
            for _ra in range(nA if "A" in phases else 0):
              with ExitStack() as actx:
                wpool = actx.enter_context(tc.tile_pool(name="wA", bufs=1))
                xpool = actx.enter_context(tc.tile_pool(name="xA", bufs=1))
                stA = actx.enter_context(tc.tile_pool(name="stA", bufs=1))
                psA = actx.enter_context(
                    tc.tile_pool(name="psA", bufs=1, space="PSUM"))

                xt_r = xt_d[:].rearrange("(hc p) n -> p hc n", p=P)

                # t=0 xt quarters interleaved with wq quarters so the first
                # accumulation group can start after one of each (~4us)
                w_tiles = {}
                for nm in ("wq", "wk", "wv"):
                    w_tiles[nm] = wpool.tile([P, HC, IN_PC], BF16, name=f"w_{nm}",
                                             tag=f"w_{nm}")
                w_tiles["wg"] = gx.tile([P, HC, IN_PC], BF16, name="w_wg",
                                        tag="w_wg")
                wq_t, wk_t, wv_t, wg_t = (w_tiles[n] for n in ("wq", "wk", "wv", "wg"))
                xq0 = [xpool.tile([P, HC // QW, CH_A], BF16, name=f"xh{i}",
                                  tag="xt", bufs=2 * QW) for i in range(QW)]
                wq_r = wq_d[:].rearrange("(hc p) m -> p hc m", p=P)
                for i in range(QW):
                    hsl = slice(i * (HC // QW), (i + 1) * (HC // QW))
                    nc.sync.dma_start(
                        xq0[i][:], xt_r[:, hsl, 0:CH_A])
                    nc.sync.dma_start(wq_t[:, hsl, :], wq_r[:, hsl, :])
                for nm, wd in (("wk", wk_d), ("wv", wv_d), ("wg", wg_d)):
                    wt = w_tiles[nm]
                    wr = wd[:].rearrange("(hc p) m -> p hc m", p=P)
                    for i in range(QW):
                        hsl = slice(i * (HC // QW), (i + 1) * (HC // QW))
                        nc.sync.dma_start(wt[:, hsl, :], wr[:, hsl, :])

                def prefetch_b_consts():
                    nc.sync.dma_start(
                        wo_t[:], wo_d[:].rearrange("(h p) n -> p h n", p=P))
                    nc.sync.dma_start(ones_t[:], ones_d[:].bitcast(FP32R))
                    nc.sync.dma_start(qdec_t[:],
                                      qdec_d[:].rearrange("h p i -> p h i"))
                    nc.sync.dma_start(dmask_t[:],
                                      dmask_d[:].rearrange("h p i -> p h i"))
                    nc.sync.dma_start(kdv_t[:], kdv_d[:])
                    nc.sync.dma_start(bdI_t[:], bdI_d[:].bitcast(FP32R))
                    nc.sync.dma_start(
                        kv_t[:],
                        kv0_d[:].rearrange("h d e -> d h e").bitcast(FP32R))

                for t in range(NT_A):
                    if t == 0:
                        xq = xq0
                    else:
                        xq = []
                        for i in range(QW):
                            if t == NT_A - 1:
                                xh = gx.tile([P, HC // QW, CH_A], BF16,
                                             name=f"xq7_{i}", tag=f"xq7_{i}")
                            else:
                                xh = xpool.tile([P, HC // QW, CH_A], BF16,
                                                tag="xt", bufs=2 * QW)
                            nc.sync.dma_start(
                                xh[:],
                                xt_r[:, i * (HC // QW):(i + 1) * (HC // QW),
                                     t * CH_A:(t + 1) * CH_A])
                            xq.append(xh)
                    if t == NT_A - 1:
                        xq7 = xq
                    if t == 1:
                        # B-phase weights + consts, behind t0/t1 input loads
                        prefetch_b_consts()

                    # q/k/g: psum[cc] += w[:,hc,cc].T @ x[hc]  (hc-outer)
                    for w_t, actf, dst, tg in (
                        (wq_t, AF.Silu, qT_d, "sq"),
                        (wk_t, AF.Silu, kT_d, "sk"),
                    ):
                        ps = [psA.tile([P, CH_A], FP32, name=f"psa{i}", tag="psA", bufs=6)
                              for i in range(4)]
                        for hc in range(HC):
                            for cc in range(4):
                                nc.tensor.matmul(
                                    ps[cc][:],
                                    w_t[:, hc, cc * P:(cc + 1) * P],
                                    xq[hc // (HC // QW)][:, hc % (HC // QW), :],
                                    start=(hc == 0), stop=(hc == HC - 1))
                        for cc in range(4):
                            sb = stA.tile([P, CH_A], BF16, tag=tg, bufs=3)
                            nc.scalar.activation(sb[:], ps[cc][:], actf)
                            nc.sync.dma_start(
                                dst[cc * P:(cc + 1) * P, t * CH_A:(t + 1) * CH_A],
                                sb[:])

                    # v: psum[t2] += x[hc][:,t2].T @ wv[hc]   (natural layout)
                    ps = [psA.tile([P, IN_PC], FP32, name=f"psv{i}", tag="psA", bufs=6)
                          for i in range(4)]
                    for hc in range(HC):
                        for t2 in range(4):
                            nc.tensor.matmul(
                                ps[t2][:],
                                xq[hc // (HC // QW)][:, hc % (HC // QW),
                                                     t2 * P:(t2 + 1) * P],
                                wv_t[:, hc, :],
                                start=(hc == 0), stop=(hc == HC - 1))
                    for t2 in range(4):
                        sb = stA.tile([P, IN_PC], BF16, tag="sv", bufs=3)
                        nc.scalar.activation(sb[:], ps[t2][:], AF.Silu)
                        nc.sync.dma_start(
                            v_d[t * CH_A + t2 * P: t * CH_A + (t2 + 1) * P, :],
                            sb[:])

                    # gate pre-activation (Copy keeps phase A on the silu
                    # table); the last chunk's sweep is deferred into phase B
                    # t=0 as PE gap fill
                    if t == NT_A - 1:
                        if "B" in phases:
                            b0 = load_b_tiles(0, eng=nc.sync)
                        continue
                    ps = [psA.tile([P, CH_A], FP32, name=f"psg{i}", tag="psA", bufs=6)
                          for i in range(4)]
                    for hc in range(HC):
                        for cc in range(4):
                            nc.tensor.matmul(
                                ps[cc][:],
                                wg_t[:, hc, cc * P:(cc + 1) * P],
                                xq[hc // (HC // QW)][:, hc % (HC // QW), :],
                                start=(hc == 0), stop=(hc == HC - 1))
                    for cc in range(4):
                        sb = stA.tile([P, CH_A], BF16, tag="sg", bufs=3)
                        nc.scalar.activation(sb[:], ps[cc][:], AF.Copy)
                        nc.sync.dma_start(
                            gT_d[cc * P:(cc + 1) * P, t * CH_A:(t + 1) * CH_A],
                            sb[:])


            # ---------------- phase B: attention + gating + out ----------------
            for _rb in range(nB if "B" in phases else 0):
              with ExitStack() as bctx:
                wk_b = bctx.enter_context(tc.tile_pool(name="wkB", bufs=1))
                psB = bctx.enter_context(
                    tc.tile_pool(name="psB", bufs=1, space="PSUM"))

                # out-projection emitters for iteration t, interleaved into
                # iteration t+1's attention to fill PE dependency-chain gaps
                def make_outproj(t, gA_t, sq_t):
                    work = []
                    for m in range(4):
                        for nt in range(4):
                            def op(m=m, nt=nt, t=t, gA_t=gA_t):
                                pso2 = psB.tile([P, 512], FP32, name="pso2",
                                                tag="ps_mix", bufs=3)
                                for h in range(HPC):
                                    nc.tensor.matmul(
                                        pso2[:],
                                        gA_t[:, h, m * P:(m + 1) * P],
                                        wo_t[:, h, nt * 512:(nt + 1) * 512],
                                        start=(h == 0), stop=(h == HPC - 1))
                                ob = wk_b.tile([P, 512], BF16, name="ob",
                                               tag="ob", bufs=8)
                                nc.scalar.copy(ob[:], pso2[:])
                                nc.sync.dma_start(
                                    pout_d[t * CH_B + m * P:
                                           t * CH_B + (m + 1) * P,
                                           nt * 512:(nt + 1) * 512], ob[:])
                            work.append(op)
                    for b in range(2):
                        def sg(b=b, t=t, sq_t=sq_t):
                            bsl = slice(b * BLOCK, (b + 1) * BLOCK)
                            pss = psB.tile([1, BLOCK], FP32, name="pss",
                                           tag="ps_mix", bufs=3)
                            for h in range(HPC):
                                nc.tensor.matmul(pss[:], ones_t[:],
                                                 sq_t[:, h, bsl],
                                                 start=(h == 0),
                                                 stop=(h == HPC - 1))
                            ssb = wk_b.tile([1, BLOCK], FP32, name="ssb",
                                            tag="ssb", bufs=2)
                            nc.scalar.copy(ssb[:], pss[:])
                            nc.sync.dma_start(
                                ssq_d[:, t * CH_B + b * BLOCK:
                                      t * CH_B + (b + 1) * BLOCK], ssb[:])
                        work.append(sg)
                    return work

                pending = []
                if "A" in phases and nA > 0:
                    tg = NT_A - 1
                    gps = {}
                    for cc in range(4):
                        for qtr in range(QW):
                            def gq(cc=cc, qtr=qtr):
                                if qtr == 0:
                                    gps[cc] = psB.tile([P, CH_A], FP32,
                                                       name=f"gps{cc}",
                                                       tag="ps_mix", bufs=3)
                                for hh in range(HC // QW):
                                    hc = qtr * (HC // QW) + hh
                                    nc.tensor.matmul(
                                        gps[cc][:],
                                        wg_t[:, hc, cc * P:(cc + 1) * P],
                                        xq7[qtr][:, hh, :],
                                        start=(hc == 0), stop=(hc == HC - 1))
                            pending.append(gq)

                        def gdrain(cc=cc):
                            sb = wk_b.tile([P, CH_A], BF16, name="sg7",
                                           tag="sg7", bufs=2)
                            nc.scalar.activation(sb[:], gps[cc][:], AF.Copy)
                            nc.sync.dma_start(
                                gT_d[cc * P:(cc + 1) * P,
                                     tg * CH_A:(tg + 1) * CH_A], sb[:])
                        pending.append(gdrain)

                def make_qd_vd(q_t, v_t):
                    qd_all = wk_b.tile([P, HPC, CH_B], FP32R, name="qd_all",
                                       tag="qd", bufs=2)
                    nc.vector.tensor_mul(qd_all[:], q_t[:], qdec_t[:])
                    vd_t = wk_b.tile([P, 4, IN_PC], BF16, name="vd_t",
                                     tag="vd", bufs=2)
                    nc.gpsimd.tensor_mul(vd_t[:, 0:2, :], v_t[:, 0:2, :],
                                         kdv_t[:, 0:2, :, :])
                    nc.gpsimd.tensor_mul(vd_t[:, 2:4, :], v_t[:, 2:4, :],
                                         kdv_t[:, 2:4, :, :])
                    return qd_all, vd_t

                nxt = (b0 if ("A" in phases and nA > 0) else load_b_tiles(0))
                nxt_dec = make_qd_vd(nxt[1], nxt[2])
                for t in range(NT_B):
                    k_t, q_t, v_t, kN_t, gs_t = nxt
                    qd_all, vd_t = nxt_dec

                    attn_t = wk_b.tile([P, HPC, CH_B], FP32R, tag="attn", bufs=2)
                    gA_t = wk_b.tile([P, HPC, CH_B], BF16, tag="gA", bufs=2)
                    sq_t = wk_b.tile([P, HPC, CH_B], FP32R, tag="sq", bufs=2)

                    # out-proj groups beyond the 16 attention fill slots run
                    # contiguously up front
                    while len(pending) > 4 * HPC:
                        pending.pop(0)()
                    for b in range(CH_B // BLOCK):
                        t0 = b * BLOCK
                        psk = psB.tile([P, HPC, P], FP32, tag="ps_kv", bufs=1)
                        for h in range(HPC):
                            hsl = slice(h * P, (h + 1) * P)
                            # scores (transposed): sT[j, i] = k_j . q_i
                            ps01 = psB.tile([P, 384], FP32, tag="ps_s", bufs=2)
                            nc.tensor.matmul(ps01[:, 0:BLOCK],
                                             k_t[:, h, t0:t0 + P],
                                             q_t[:, h, t0:t0 + BLOCK],
                                             start=True, stop=True)
                            nc.tensor.matmul(ps01[:, BLOCK:384],
                                             k_t[:, h, t0 + P:t0 + BLOCK],
                                             q_t[:, h, t0 + P:t0 + BLOCK],
                                             start=True, stop=True)
                            s01 = wk_b.tile([P, 384], BF16, tag="s01", bufs=2)
                            nc.vector.tensor_mul(s01[:], ps01[:],
                                                 dmask_t[:, h, :])
                            # attention output (transposed): inter + intra
                            pso = psB.tile([P, BLOCK], FP32, tag="ps_o", bufs=2)
                            nc.tensor.matmul(pso[:], kv_t[:, h, :], qd_all[:, h, t0:t0 + BLOCK],
                                             start=True, stop=False)
                            nc.tensor.matmul(pso[:], v_t[:, 2 * b, hsl],
                                             s01[:, 0:BLOCK],
                                             start=False, stop=False)
                            nc.tensor.matmul(pso[:, P:BLOCK],
                                             v_t[:, 2 * b + 1, hsl],
                                             s01[:, BLOCK:384],
                                             start=False, stop=True)
                            nc.scalar.copy(attn_t[:, h, t0:t0 + BLOCK], pso[:])
                            if pending:
                                pending.pop(0)()
                            # kv delta: k^T (kdec*v) + bd*kv into the psk bank
                            nc.tensor.matmul(psk[:, h, :], kN_t[:, 2 * b, hsl],
                                             vd_t[:, 2 * b, hsl],
                                             start=True, stop=False)
                            nc.tensor.matmul(psk[:, h, :], kN_t[:, 2 * b + 1, hsl],
                                             vd_t[:, 2 * b + 1, hsl],
                                             start=False, stop=False)
                            nc.tensor.matmul(psk[:, h, :], bdI_t[:, h, :],
                                             kv_t[:, h, :],
                                             start=False, stop=True)
                            # fill the PE chain-latency gap with prior-t
                            # out-projection groups (spread all of them
                            # across this iteration's head slots)
                            if pending:
                                pending.pop(0)()
                        # kv <- bd*kv + delta (already combined in PSUM); ACT's
                        # queue is empty at block end, DVE's is not
                        nc.scalar.copy(kv_t[:], psk[:])
                        # gating + squares for this half (overlaps next block)
                        bsl = slice(t0, t0 + BLOCK)
                        nc.vector.tensor_mul(gA_t[:, :, bsl], attn_t[:, :, bsl],
                                             gs_t[:, :, bsl])
                        nc.gpsimd.tensor_mul(sq_t[:, :, bsl], attn_t[:, :, bsl],
                                             attn_t[:, :, bsl])
                        if b == 0 and t + 1 < NT_B:
                            nxt = load_b_tiles(t + 1)

                    if t + 1 < NT_B:
                        nxt_dec = make_qd_vd(nxt[1], nxt[2])
                    while pending:
                        pending.pop(0)()
                    pending = make_outproj(t, gA_t, sq_t)

                # epilogue: last iteration's out projection
                while pending:
                    pending.pop(0)()

        if timing:
            tb = const.tile([1, 4], FP32)
            nc.vector.memset(tb[:], 0.0)
            nc.sync.dma_start(tout_d[:], tb[:])

    nc.compile()
    return nc


_NC_CACHE = {}


def _get_nc(repeat=1, phases="AB", nA=1, nB=1, timing=False):
    key = (repeat, phases, nA, nB, timing)
    if key not in _NC_CACHE:
        _NC_CACHE[key] = build_nc(repeat, phases, nA, nB, timing)
    return _NC_CACHE[key]


def make_in_maps(inputs):
    BF = ml_dtypes.bfloat16
    hs = np.ascontiguousarray(np.asarray(inputs["hidden_states"], dtype=np.float32))
    w_qkv = np.asarray(inputs["w_qkv"], dtype=np.float32)
    w_gate = np.asarray(inputs["w_gate"], dtype=np.float32)
    w_out = np.asarray(inputs["w_out"], dtype=np.float32)
    norm_weight = np.asarray(inputs["norm_weight"], dtype=np.float32)
    slope_rate = np.asarray(inputs["slope_rate"], dtype=np.float32).reshape(NUM_HEADS)
    kv_cache = np.asarray(inputs["kv_cache"], dtype=np.float32)

    xt = np.ascontiguousarray(hs.T).astype(BF)           # [HIDDEN, SEQ]
    wq3 = w_qkv.reshape(HIDDEN, NUM_HEADS, 3 * HEAD_DIM)
    ones = np.ones((P, 1), dtype=np.float32)
    idx = np.arange(BLOCK, dtype=np.float64)

    in_maps = []
    for c in range(N_CORES):
        s = slope_rate[c * HPC:(c + 1) * HPC].astype(np.float64)  # [HPC]
        wq = np.ascontiguousarray(
            wq3[:, c * HPC:(c + 1) * HPC, 0:HEAD_DIM].reshape(HIDDEN, IN_PC)).astype(BF)
        wk = np.ascontiguousarray(
            wq3[:, c * HPC:(c + 1) * HPC, HEAD_DIM:2 * HEAD_DIM].reshape(HIDDEN, IN_PC)).astype(BF)
        wv = np.ascontiguousarray(
            wq3[:, c * HPC:(c + 1) * HPC, 2 * HEAD_DIM:3 * HEAD_DIM].reshape(HIDDEN, IN_PC)).astype(BF)
        wg = np.ascontiguousarray(w_gate[:, c * IN_PC:(c + 1) * IN_PC]).astype(BF)
        nw = norm_weight[c * IN_PC:(c + 1) * IN_PC]
        wo = np.ascontiguousarray(
            nw[:, None] * w_out[c * IN_PC:(c + 1) * IN_PC, :]).astype(BF)

        # dmask0[h, j, i] = exp(-s (i - j)) for i >= j (j in 0..127, i in 0..255)
        jj = idx[:128][:, None]                          # [128,1]
        ii = idx[None, :]                                # [1,256]
        d0 = np.exp(-s[:, None, None] * (ii - jj)) * (ii >= jj)
        dmask0 = d0.astype(np.float32)                   # [HPC,128,256]
        # dmask1[h, j', i] for abs j = j'+128: zero for i<128, else dmask0[j', i-128]
        dmask1 = np.zeros((HPC, P, BLOCK), dtype=np.float32)
        dmask1[:, :, P:] = dmask0[:, :, :P]
        dmask = np.concatenate([dmask0, dmask1[:, :, P:]], axis=2).astype(BF)
        qdec1 = np.exp(-s[:, None] * (idx[None, :] + 1.0))      # [HPC, 256]
        qdec = np.broadcast_to(
            np.concatenate([qdec1, qdec1], axis=1)[:, None, :],
            (HPC, P, CH_B)).astype(BF)
        kdec = np.exp(-s[:, None] * (BLOCK - 1.0 - idx[None, :]))  # [HPC, 256]
        # kdv[p, s, h, d] = kdec[h, (s % 2)*128 + p], broadcast over d
        kdv = np.zeros((P, 4, HPC, P), dtype=BF)
        for sb_ in range(4):
            kdv[:, sb_, :, :] = kdec[:, (sb_ % 2) * P:(sb_ % 2) * P + P].T[:, :, None].astype(BF)
        bdv = np.exp(-s * BLOCK).astype(np.float32)              # [HPC]
        bdI = np.zeros((P, HPC, P), dtype=np.float32)
        for hh in range(HPC):
            np.fill_diagonal(bdI[:, hh, :], bdv[hh])
        kv0 = np.ascontiguousarray(kv_cache[c * HPC:(c + 1) * HPC])

        in_maps.append({
            "xt": xt, "wq": wq, "wk": wk, "wv": wv, "wg": wg, "wo": wo,
            "qdec": np.ascontiguousarray(qdec),
            "dmask": np.ascontiguousarray(dmask), "kdv": kdv,
            "bdI": bdI, "ones": ones,
            "kv0": kv0,
        })
    return in_maps


def combine_outputs(results):
    pout = np.zeros((SEQ, HIDDEN), dtype=np.float64)
    ssq = np.zeros((SEQ,), dtype=np.float64)
    for r in results:
        pout += r["pout"].astype(np.float64)
        ssq += r["ssq"].reshape(SEQ).astype(np.float64)
    var = ssq / INNER
    scale = 1.0 / np.sqrt(var + EPS)
    return (pout * scale[:, None]).astype(np.float32)


def kernel(**inputs):
    nc = _get_nc(1)
    in_maps = make_in_maps(inputs)
    res = run_bass_kernel_spmd(nc, in_maps, core_ids=list(range(N_CORES)))
    return combine_outputs(res.results)


# revision 40
# speedup vs baseline: 1.0402x; 1.0402x over previous
"""MiniMaxText01 linear attention layer on 8 trn2 NeuronCores.

Strategy: tensor-parallel over heads (4 heads/core). Per core, two phases
through internal DRAM (SBUF can't hold all weights + full-seq activations):

  phase A: qT/kT/v/gT projections as bf16 matmuls (full PE rate), silu on
           ACT for q/k/v, gate stored as PRE-activation (Copy) so phase A
           only ever uses the silu act-table and phase B only the sigmoid
           table (one table switch per kernel, not per iteration).
           hc-outer accumulation order with 4 PSUM groups in flight so the
           first matmul starts ~3us in, paced by the streaming weight DMAs.
  phase B: blocked lightning attention (BLOCK=256) + sigmoid gating + out
           projection. kv state [d,128] per head stays fp32r in SBUF; all
           big matmul operands are bf16 (rel err ~3e-3 vs 2e-2 budget).
           The RMSNorm rsqrt(var) is a per-token scalar that commutes with
           the out projection, so each core emits
             pout = (gate * attn * norm_w) @ w_out        [4096, 2048]
             ssq  = sum over this core's channels of attn^2   [1, 4096]
           and the host applies out = sum_c(pout) * rsqrt(sum_c(ssq)/4096+eps).
"""
import math
import numpy as np
import ml_dtypes
from contextlib import ExitStack

import concourse.bass as bass
import concourse.tile as tile
import concourse.mybir as mybir
from concourse import bacc
from concourse.bass_utils import run_bass_kernel_spmd

FP32 = mybir.dt.float32
FP32R = mybir.dt.float32r
BF16 = mybir.dt.bfloat16
AF = mybir.ActivationFunctionType

SEQ = 4096
HIDDEN = 2048
NUM_HEADS = 32
HEAD_DIM = 128
INNER = NUM_HEADS * HEAD_DIM
BLOCK = 256
EPS = 1e-5
N_CORES = 8
HPC = NUM_HEADS // N_CORES          # 4 heads per core
IN_PC = HPC * HEAD_DIM              # 512 inner channels per core
P = 128

CH_A = 512
NT_A = SEQ // CH_A                  # 8
CH_B = 512
NT_B = SEQ // CH_B                  # 8
HC = HIDDEN // P                    # 16 hidden chunks
QW = 4                              # xt/weight quarters per chunk


def build_nc(repeat: int = 1, phases: str = "AB", nA: int = 1, nB: int = 1,
             timing: bool = False):
    nc = bacc.Bacc("TRN2", target_bir_lowering=False)

    xt_d = nc.dram_tensor("xt", [HIDDEN, SEQ], BF16, kind="ExternalInput")
    wq_d = nc.dram_tensor("wq", [HIDDEN, IN_PC], BF16, kind="ExternalInput")
    wk_d = nc.dram_tensor("wk", [HIDDEN, IN_PC], BF16, kind="ExternalInput")
    wv_d = nc.dram_tensor("wv", [HIDDEN, IN_PC], BF16, kind="ExternalInput")
    wg_d = nc.dram_tensor("wg", [HIDDEN, IN_PC], BF16, kind="ExternalInput")
    wo_d = nc.dram_tensor("wo", [IN_PC, HIDDEN], BF16, kind="ExternalInput")
    qdec_d = nc.dram_tensor("qdec", [HPC, P, CH_B], BF16, kind="ExternalInput")
    dmask_d = nc.dram_tensor("dmask", [HPC, P, 384], BF16, kind="ExternalInput")
    kdv_d = nc.dram_tensor("kdv", [P, 4, HPC, P], BF16, kind="ExternalInput")
    bdI_d = nc.dram_tensor("bdI", [P, HPC, P], FP32, kind="ExternalInput")
    ones_d = nc.dram_tensor("ones", [P, 1], FP32, kind="ExternalInput")
    kv0_d = nc.dram_tensor("kv0", [HPC, P, P], FP32, kind="ExternalInput")

    # timing builds keep the full compute + DMA but only ship a tiny output
    okind = "Internal" if timing else "ExternalOutput"
    pout_d = nc.dram_tensor("pout", [SEQ, HIDDEN], BF16, kind=okind)
    ssq_d = nc.dram_tensor("ssq", [1, SEQ], FP32, kind=okind)
    if timing:
        tout_d = nc.dram_tensor("tout", [1, 4], FP32, kind="ExternalOutput")

    # internal DRAM intermediates (bf16 round trip)
    qT_d = nc.dram_tensor("qT_i", [IN_PC, SEQ], BF16)
    kT_d = nc.dram_tensor("kT_i", [IN_PC, SEQ], BF16)
    v_d = nc.dram_tensor("v_i", [SEQ, IN_PC], BF16)
    gT_d = nc.dram_tensor("gT_i", [IN_PC, SEQ], BF16)   # gate PRE-activation

    with tile.TileContext(nc) as tc, ExitStack() as ctx:
        const = ctx.enter_context(tc.tile_pool(name="const", bufs=1))
        sB = ctx.enter_context(tc.tile_pool(name="sB", bufs=1))
        gx = ctx.enter_context(tc.tile_pool(name="gx", bufs=1))

        qT_r = qT_d[:].rearrange("(h p) n -> p h n", p=P)
        kT_r = kT_d[:].rearrange("(h p) n -> p h n", p=P)
        gT_r = gT_d[:].rearrange("(h p) n -> p h n", p=P)

        def load_b_tiles(t, eng=None):
            eng = eng or nc.scalar
            tsl = slice(t * CH_B, (t + 1) * CH_B)
            k_t = sB.tile([P, HPC, CH_B], BF16, name="k_t", tag="k", bufs=2)
            eng.dma_start(k_t[:], kT_r[:, :, tsl])
            q_t = sB.tile([P, HPC, CH_B], BF16, name="q_t", tag="q", bufs=2)
            eng.dma_start(q_t[:], qT_r[:, :, tsl])
            v_t = sB.tile([P, 4, IN_PC], BF16, name="v_t", tag="v", bufs=2)
            eng.dma_start(
                v_t[:], v_d[tsl, :].rearrange("(s p) c -> p s c", p=P))
            # k in natural [token, channel] layout via xbar transpose DMA
            kN_t = sB.tile([P, 4, IN_PC], BF16, name="kN_t", tag="kN", bufs=2)
            for s in range(4):
                eng.dma_start_transpose(
                    kN_t[:, s, :],
                    kT_d[:, t * CH_B + s * P: t * CH_B + (s + 1) * P])
            g_t = sB.tile([P, HPC, CH_B], BF16, name="g_t", tag="g", bufs=2)
            eng.dma_start(g_t[:], gT_r[:, :, tsl])
            gs_t = sB.tile([P, HPC, CH_B], BF16, name="gs_t", tag="gs", bufs=2)
            nc.scalar.activation(gs_t[:], g_t[:], AF.Sigmoid)
            return k_t, q_t, v_t, kN_t, gs_t

        ones_t = const.tile([P, 1], FP32R)
        qdec_t = const.tile([P, HPC, CH_B], BF16)
        dmask_t = const.tile([P, HPC, 384], BF16)
        kdv_t = const.tile([P, 4, HPC, P], BF16)
        bdI_t = const.tile([P, HPC, P], FP32R)
        kv_t = const.tile([P, HPC, P], FP32R)
        wo_t = const.tile([P, HPC, HIDDEN], BF16)

        for _rep in range(repeat):
            # ---------------- phase A: projections ----------------
            for _ra in range(nA if "A" in phases else 0):
              with ExitStack() as actx:
                wpool = actx.enter_context(tc.tile_pool(name="wA", bufs=1))
                xpool = actx.enter_context(tc.tile_pool(name="xA", bufs=1))
                stA = actx.enter_context(tc.tile_pool(name="stA", bufs=1))
                psA = actx.enter_context(
                    tc.tile_pool(name="psA", bufs=1, space="PSUM"))

                xt_r = xt_d[:].rearrange("(hc p) n -> p hc n", p=P)

                # t=0 xt quarters interleaved with wq quarters so the first
                # accumulation group can start after one of each (~4us)
                w_tiles = {}
                for nm in ("wq", "wk", "wv"):
                    w_tiles[nm] = wpool.tile([P, HC, IN_PC], BF16, name=f"w_{nm}",
                                             tag=f"w_{nm}")
                w_tiles["wg"] = gx.tile([P, HC, IN_PC], BF16, name="w_wg",
                                        tag="w_wg")
                wq_t, wk_t, wv_t, wg_t = (w_tiles[n] for n in ("wq", "wk", "wv", "wg"))
                xq0 = [xpool.tile([P, HC // QW, CH_A], BF16, name=f"xh{i}",
                                  tag="xt", bufs=2 * QW) for i in range(QW)]
                wq_r = wq_d[:].rearrange("(hc p) m -> p hc m", p=P)
                for i in range(QW):
                    hsl = slice(i * (HC // QW), (i + 1) * (HC // QW))
                    nc.sync.dma_start(
                        xq0[i][:], xt_r[:, hsl, 0:CH_A])
                    nc.sync.dma_start(wq_t[:, hsl, :], wq_r[:, hsl, :])
                for nm, wd in (("wk", wk_d), ("wv", wv_d), ("wg", wg_d)):
                    wt = w_tiles[nm]
                    wr = wd[:].rearrange("(hc p) m -> p hc m", p=P)
                    for i in range(QW):
                        hsl = slice(i * (HC // QW), (i + 1) * (HC // QW))
                        nc.sync.dma_start(wt[:, hsl, :], wr[:, hsl, :])

                def prefetch_b_consts():
                    nc.sync.dma_start(
                        wo_t[:], wo_d[:].rearrange("(h p) n -> p h n", p=P))
                    nc.sync.dma_start(ones_t[:], ones_d[:].bitcast(FP32R))
                    nc.sync.dma_start(qdec_t[:],
                                      qdec_d[:].rearrange("h p i -> p h i"))
                    nc.sync.dma_start(dmask_t[:],
                                      dmask_d[:].rearrange("h p i -> p h i"))
                    nc.sync.dma_start(kdv_t[:], kdv_d[:])
                    nc.sync.dma_start(bdI_t[:], bdI_d[:].bitcast(FP32R))
                    nc.sync.dma_start(
                        kv_t[:],
                        kv0_d[:].rearrange("h d e -> d h e").bitcast(FP32R))

                for t in range(NT_A):
                    if t == 0:
                        xq = xq0
                    else:
                        xq = []
                        for i in range(QW):
                            if t == NT_A - 1:
                                xh = gx.tile([P, HC // QW, CH_A], BF16,
                                             name=f"xq7_{i}", tag=f"xq7_{i}")
                            else:
                                xh = xpool.tile([P, HC // QW, CH_A], BF16,
                                                tag="xt", bufs=2 * QW)
                            nc.sync.dma_start(
                                xh[:],
                                xt_r[:, i * (HC // QW):(i + 1) * (HC // QW),
                                     t * CH_A:(t + 1) * CH_A])
                            xq.append(xh)
                    if t == NT_A - 1:
                        xq7 = xq
                    if t == 1:
                        # B-phase weights + consts, behind t0/t1 input loads
                        prefetch_b_consts()

                    # q/k/g: psum[cc] += w[:,hc,cc].T @ x[hc]  (hc-outer)
                    for w_t, actf, dst, tg in (
                        (wq_t, AF.Silu, qT_d, "sq"),
                        (wk_t, AF.Silu, kT_d, "sk"),
                    ):
                        ps = [psA.tile([P, CH_A], FP32, name=f"psa{i}", tag="psA", bufs=6)
                              for i in range(4)]
                        for hc in range(HC):
                            for cc in range(4):
                                nc.tensor.matmul(
                                    ps[cc][:],
                                    w_t[:, hc, cc * P:(cc + 1) * P],
                                    xq[hc // (HC // QW)][:, hc % (HC // QW), :],
                                    start=(hc == 0), stop=(hc == HC - 1))
                        for cc in range(4):
                            sb = stA.tile([P, CH_A], BF16, tag=tg, bufs=3)
                            nc.scalar.activation(sb[:], ps[cc][:], actf)
                            nc.sync.dma_start(
                                dst[cc * P:(cc + 1) * P, t * CH_A:(t + 1) * CH_A],
                                sb[:])

                    # v: psum[t2] += x[hc][:,t2].T @ wv[hc]   (natural layout)
                    ps = [psA.tile([P, IN_PC], FP32, name=f"psv{i}", tag="psA", bufs=6)
                          for i in range(4)]
                    for hc in range(HC):
                        for t2 in range(4):
                            nc.tensor.matmul(
                                ps[t2][:],
                                xq[hc // (HC // QW)][:, hc % (HC // QW),
                                                     t2 * P:(t2 + 1) * P],
                                wv_t[:, hc, :],
                                start=(hc == 0), stop=(hc == HC - 1))
                    for t2 in range(4):
                        sb = stA.tile([P, IN_PC], BF16, tag="sv", bufs=3)
                        nc.scalar.activation(sb[:], ps[t2][:], AF.Silu)
                        nc.sync.dma_start(
                            v_d[t * CH_A + t2 * P: t * CH_A + (t2 + 1) * P, :],
                            sb[:])

                    # gate pre-activation (Copy keeps phase A on the silu
                    # table); the last chunk's sweep is deferred into phase B
                    # t=0 as PE gap fill
                    if t == NT_A - 1:
                        if "B" in phases:
                            b0 = load_b_tiles(0, eng=nc.sync)
                        continue
                    ps = [psA.tile([P, CH_A], FP32, name=f"psg{i}", tag="psA", bufs=6)
                          for i in range(4)]
                    for hc in range(HC):
                        for cc in range(4):
                            nc.tensor.matmul(
                                ps[cc][:],
                                wg_t[:, hc, cc * P:(cc + 1) * P],
                                xq[hc // (HC // QW)][:, hc % (HC // QW), :],
                                start=(hc == 0), stop=(hc == HC - 1))
                    for cc in range(4):
                        sb = stA.tile([P, CH_A], BF16, tag="sg", bufs=3)
                        nc.scalar.activation(sb[:], ps[cc][:], AF.Copy)
                        nc.sync.dma_start(
                            gT_d[cc * P:(cc + 1) * P, t * CH_A:(t + 1) * CH_A],
                            sb[:])


            # ---------------- phase B: attention + gating + out ----------------
            for _rb in range(nB if "B" in phases else 0):
              with ExitStack() as bctx:
                wk_b = bctx.enter_context(tc.tile_pool(name="wkB", bufs=1))
                psB = bctx.enter_context(
                    tc.tile_pool(name="psB", bufs=1, space="PSUM"))

                # out-projection emitters for iteration t, interleaved into
                # iteration t+1's attention to fill PE dependency-chain gaps
                def make_outproj(t, gA_t, sq_t):
                    work = []
                    for m in range(4):
                        for nt in range(4):
                            def op(m=m, nt=nt, t=t, gA_t=gA_t):
                                pso2 = psB.tile([P, 512], FP32, name="pso2",
                                                tag="ps_mix", bufs=3)
                                for h in range(HPC):
                                    nc.tensor.matmul(
                                        pso2[:],
                                        gA_t[:, h, m * P:(m + 1) * P],
                                        wo_t[:, h, nt * 512:(nt + 1) * 512],
                                        start=(h == 0), stop=(h == HPC - 1))
                                ob = wk_b.tile([P, 512], BF16, name="ob",
                                               tag="ob", bufs=8)
                                nc.scalar.copy(ob[:], pso2[:])
                                nc.sync.dma_start(
                                    pout_d[t * CH_B + m * P:
                                           t * CH_B + (m + 1) * P,
                                           nt * 512:(nt + 1) * 512], ob[:])
                            work.append(op)
                    for b in range(2):
                        def sg(b=b, t=t, sq_t=sq_t):
                            bsl = slice(b * BLOCK, (b + 1) * BLOCK)
                            pss = psB.tile([1, BLOCK], FP32, name="pss",
                                           tag="ps_mix", bufs=3)
                            for h in range(HPC):
                                nc.tensor.matmul(pss[:], ones_t[:],
                                                 sq_t[:, h, bsl],
                                                 start=(h == 0),
                                                 stop=(h == HPC - 1))
                            ssb = wk_b.tile([1, BLOCK], FP32, name="ssb",
                                            tag="ssb", bufs=2)
                            nc.scalar.copy(ssb[:], pss[:])
                            nc.sync.dma_start(
                                ssq_d[:, t * CH_B + b * BLOCK:
                                      t * CH_B + (b + 1) * BLOCK], ssb[:])
                        work.append(sg)
                    return work

                pending = []
                if "A" in phases and nA > 0:
                    tg = NT_A - 1
                    gps = {}
                    for cc in range(4):
                        for qtr in range(QW):
                            def gq(cc=cc, qtr=qtr):
                                if qtr == 0:
                                    gps[cc] = psB.tile([P, CH_A], FP32,
                                                       name=f"gps{cc}",
                                                       tag="ps_mix", bufs=3)
                                for hh in range(HC // QW):
                                    hc = qtr * (HC // QW) + hh
                                    nc.tensor.matmul(
                                        gps[cc][:],
                                        wg_t[:, hc, cc * P:(cc + 1) * P],
                                        xq7[qtr][:, hh, :],
                                        start=(hc == 0), stop=(hc == HC - 1))
                            pending.append(gq)

                        def gdrain(cc=cc):
                            sb = wk_b.tile([P, CH_A], BF16, name="sg7",
                                           tag="sg7", bufs=2)
                            nc.scalar.activation(sb[:], gps[cc][:], AF.Copy)
                            nc.sync.dma_start(
                                gT_d[cc * P:(cc + 1) * P,
                                     tg * CH_A:(tg + 1) * CH_A], sb[:])
                        pending.append(gdrain)

                def make_qd_vd(q_t, v_t):
                    qd_all = wk_b.tile([P, HPC, CH_B], FP32R, name="qd_all",
                                       tag="qd", bufs=2)
                    nc.vector.tensor_mul(qd_all[:], q_t[:], qdec_t[:])
                    vd_t = wk_b.tile([P, 4, IN_PC], BF16, name="vd_t",
                                     tag="vd", bufs=2)
                    nc.gpsimd.tensor_mul(vd_t[:, 0:2, :], v_t[:, 0:2, :],
                                         kdv_t[:, 0:2, :, :])
                    nc.gpsimd.tensor_mul(vd_t[:, 2:4, :], v_t[:, 2:4, :],
                                         kdv_t[:, 2:4, :, :])
                    return qd_all, vd_t

                nxt = (b0 if ("A" in phases and nA > 0) else load_b_tiles(0))
                nxt_dec = make_qd_vd(nxt[1], nxt[2])
                for t in range(NT_B):
                    k_t, q_t, v_t, kN_t, gs_t = nxt
                    qd_all, vd_t = nxt_dec

                    attn_t = wk_b.tile([P, HPC, CH_B], FP32R, tag="attn", bufs=2)
                    gA_t = wk_b.tile([P, HPC, CH_B], BF16, tag="gA", bufs=2)
                    sq_t = wk_b.tile([P, HPC, CH_B], FP32R, tag="sq", bufs=2)

                    # out-proj groups beyond the 16 attention fill slots run
                    # contiguously up front
                    while len(pending) > 4 * HPC:
                        pending.pop(0)()
                    for b in range(CH_B // BLOCK):
                        t0 = b * BLOCK
                        psk = psB.tile([P, HPC, P], FP32, tag="ps_kv", bufs=1)
                        for h in range(HPC):
                            hsl = slice(h * P, (h + 1) * P)
                            # scores (transposed): sT[j, i] = k_j . q_i
                            ps01 = psB.tile([P, 384], FP32, tag="ps_s", bufs=2)
                            nc.tensor.matmul(ps01[:, 0:BLOCK],
                                             k_t[:, h, t0:t0 + P],
                                             q_t[:, h, t0:t0 + BLOCK],
                                             start=True, stop=True)
                            nc.tensor.matmul(ps01[:, BLOCK:384],
                                             k_t[:, h, t0 + P:t0 + BLOCK],
                                             q_t[:, h, t0 + P:t0 + BLOCK],
                                             start=True, stop=True)
                            s01 = wk_b.tile([P, 384], BF16, tag="s01", bufs=2)
                            nc.vector.tensor_mul(s01[:], ps01[:],
                                                 dmask_t[:, h, :])
                            # attention output (transposed): inter + intra
                            pso = psB.tile([P, BLOCK], FP32, tag="ps_o", bufs=2)
                            nc.tensor.matmul(pso[:], kv_t[:, h, :], qd_all[:, h, t0:t0 + BLOCK],
                                             start=True, stop=False)
                            nc.tensor.matmul(pso[:], v_t[:, 2 * b, hsl],
                                             s01[:, 0:BLOCK],
                                             start=False, stop=False)
                            nc.tensor.matmul(pso[:, P:BLOCK],
                                             v_t[:, 2 * b + 1, hsl],
                                             s01[:, BLOCK:384],
                                             start=False, stop=True)
                            nc.scalar.copy(attn_t[:, h, t0:t0 + BLOCK], pso[:])
                            if pending:
                                pending.pop(0)()
                            # kv delta: k^T (kdec*v) + bd*kv into the psk bank
                            nc.tensor.matmul(psk[:, h, :], kN_t[:, 2 * b, hsl],
                                             vd_t[:, 2 * b, hsl],
                                             start=True, stop=False)
                            nc.tensor.matmul(psk[:, h, :], kN_t[:, 2 * b + 1, hsl],
                                             vd_t[:, 2 * b + 1, hsl],
                                             start=False, stop=False)
                            nc.tensor.matmul(psk[:, h, :], bdI_t[:, h, :],
                                             kv_t[:, h, :],
                                             start=False, stop=True)
                            # fill the PE chain-latency gap with prior-t
                            # out-projection groups (spread all of them
                            # across this iteration's head slots)
                            if pending:
                                pending.pop(0)()
                        # kv <- bd*kv + delta (already combined in PSUM); ACT's
                        # queue is empty at block end, DVE's is not
                        nc.scalar.copy(kv_t[:], psk[:])
                        # gating + squares for this half (overlaps next block)
                        bsl = slice(t0, t0 + BLOCK)
                        nc.vector.tensor_mul(gA_t[:, :, bsl], attn_t[:, :, bsl],
                                             gs_t[:, :, bsl])
                        nc.gpsimd.tensor_mul(sq_t[:, :, bsl], attn_t[:, :, bsl],
                                             attn_t[:, :, bsl])
                        if b == 0 and t + 1 < NT_B:
                            nxt = load_b_tiles(t + 1)

                    if t + 1 < NT_B:
                        nxt_dec = make_qd_vd(nxt[1], nxt[2])
                    while pending:
                        pending.pop(0)()
                    pending = make_outproj(t, gA_t, sq_t)

                # epilogue: last iteration's out projection
                while pending:
                    pending.pop(0)()

        if timing:
            tb = const.tile([1, 4], FP32)
            nc.vector.memset(tb[:], 0.0)
            nc.sync.dma_start(tout_d[:], tb[:])

    nc.compile()
    return nc


_NC_CACHE = {}


def _get_nc(repeat=1, phases="AB", nA=1, nB=1, timing=False):
    key = (repeat, phases, nA, nB, timing)
    if key not in _NC_CACHE:
        _NC_CACHE[key] = build_nc(repeat, phases, nA, nB, timing)
    return _NC_CACHE[key]


def make_in_maps(inputs):
    BF = ml_dtypes.bfloat16
    hs = np.ascontiguousarray(np.asarray(inputs["hidden_states"], dtype=np.float32))
    w_qkv = np.asarray(inputs["w_qkv"], dtype=np.float32)
    w_gate = np.asarray(inputs["w_gate"], dtype=np.float32)
    w_out = np.asarray(inputs["w_out"], dtype=np.float32)
    norm_weight = np.asarray(inputs["norm_weight"], dtype=np.float32)
    slope_rate = np.asarray(inputs["slope_rate"], dtype=np.float32).reshape(NUM_HEADS)
    kv_cache = np.asarray(inputs["kv_cache"], dtype=np.float32)

    xt = np.ascontiguousarray(hs.T).astype(BF)           # [HIDDEN, SEQ]
    wq3 = w_qkv.reshape(HIDDEN, NUM_HEADS, 3 * HEAD_DIM)
    ones = np.ones((P, 1), dtype=np.float32)
    idx = np.arange(BLOCK, dtype=np.float64)

    in_maps = []
    for c in range(N_CORES):
        s = slope_rate[c * HPC:(c + 1) * HPC].astype(np.float64)  # [HPC]
        wq = np.ascontiguousarray(
            wq3[:, c * HPC:(c + 1) * HPC, 0:HEAD_DIM].reshape(HIDDEN, IN_PC)).astype(BF)
        wk = np.ascontiguousarray(
            wq3[:, c * HPC:(c + 1) * HPC, HEAD_DIM:2 * HEAD_DIM].reshape(HIDDEN, IN_PC)).astype(BF)
        wv = np.ascontiguousarray(
            wq3[:, c * HPC:(c + 1) * HPC, 2 * HEAD_DIM:3 * HEAD_DIM].reshape(HIDDEN, IN_PC)).astype(BF)
        wg = np.ascontiguousarray(w_gate[:, c * IN_PC:(c + 1) * IN_PC]).astype(BF)
        nw = norm_weight[c * IN_PC:(c + 1) * IN_PC]
        wo = np.ascontiguousarray(
            nw[:, None] * w_out[c * IN_PC:(c + 1) * IN_PC, :]).astype(BF)

        # dmask0[h, j, i] = exp(-s (i - j)) for i >= j (j in 0..127, i in 0..255)
        jj = idx[:128][:, None]                          # [128,1]
        ii = idx[None, :]                                # [1,256]
        d0 = np.exp(-s[:, None, None] * (ii - jj)) * (ii >= jj)
        dmask0 = d0.astype(np.float32)                   # [HPC,128,256]
        # dmask1[h, j', i] for abs j = j'+128: zero for i<128, else dmask0[j', i-128]
        dmask1 = np.zeros((HPC, P, BLOCK), dtype=np.float32)
        dmask1[:, :, P:] = dmask0[:, :, :P]
        dmask = np.concatenate([dmask0, dmask1[:, :, P:]], axis=2).astype(BF)
        qdec1 = np.exp(-s[:, None] * (idx[None, :] + 1.0))      # [HPC, 256]
        qdec = np.broadcast_to(
            np.concatenate([qdec1, qdec1], axis=1)[:, None, :],
            (HPC, P, CH_B)).astype(BF)
        kdec = np.exp(-s[:, None] * (BLOCK - 1.0 - idx[None, :]))  # [HPC, 256]
        # kdv[p, s, h, d] = kdec[h, (s % 2)*128 + p], broadcast over d
        kdv = np.zeros((P, 4, HPC, P), dtype=BF)
        for sb_ in range(4):
            kdv[:, sb_, :, :] = kdec[:, (sb_ % 2) * P:(sb_ % 2) * P + P].T[:, :, None].astype(BF)
        bdv = np.exp(-s * BLOCK).astype(np.float32)              # [HPC]
        bdI = np.zeros((P, HPC, P), dtype=np.float32)
        for hh in range(HPC):
            np.fill_diagonal(bdI[:, hh, :], bdv[hh])
        kv0 = np.ascontiguousarray(kv_cache[c * HPC:(c + 1) * HPC])

        in_maps.append({
            "xt": xt, "wq": wq, "wk": wk, "wv": wv, "wg": wg, "wo": wo,
            "qdec": np.ascontiguousarray(qdec),
            "dmask": np.ascontiguousarray(dmask), "kdv": kdv,
            "bdI": bdI, "ones": ones,
            "kv0": kv0,
        })
    return in_maps


def combine_outputs(results):
    pout = np.zeros((SEQ, HIDDEN), dtype=np.float64)
    ssq = np.zeros((SEQ,), dtype=np.float64)
    for r in results:
        pout += r["pout"].astype(np.float64)
        ssq += r["ssq"].reshape(SEQ).astype(np.float64)
    var = ssq / INNER
    scale = 1.0 / np.sqrt(var + EPS)
    return (pout * scale[:, None]).astype(np.float32)


def kernel(**inputs):
    nc = _get_nc(1)
    in_maps = make_in_maps(inputs)
    res = run_bass_kernel_spmd(nc, in_maps, core_ids=list(range(N_CORES)))
    return combine_outputs(res.results)
